# revision 6
# baseline (speedup 1.0000x reference)
"""AdaptDHM MoE-routing kernel for one TRN2 chip (8 NeuronCores).

Strategy (expert-parallel dispatch, done host-side):
  - router = argmax(x @ center.T) picks one of C=8 clusters per token.
  - The reference computes ALL 8 cluster towers for every token and then
    gathers the selected one; only 1/8 of that work is observable. We
    dispatch each token to the core owning its cluster and run the
    4-layer MLP (1024->2048->1024->512->1, relu/sigmoid) once per token.
  - Core d receives the tokens routed to cluster d, padded to a common
    capacity K (SPMD: all cores run the same NEFF), plus the gated
    weights w0_l * wc_l[d] pre-cast to bf16.
  - On-device: feature-major layout ([feature, token]) throughout, weights
    are the stationary matmul operand, activations the moving operand,
    fp32 PSUM accumulation, relu/sigmoid fused on the Scalar engine.
  - Host scatters per-core results back to the [B] output.
"""

import math
import os

import ml_dtypes
import numpy as np

B, DIMS = 8192, 1024
FCN = [DIMS, 2048, 1024, 512, 1]
C = 8
NCORES = 8
P = 128
TT = 512  # token tile (matmul moving free dim / PSUM bank)

_BF16 = ml_dtypes.bfloat16

_graph_cache = {}
last_run = None  # BassKernelResults of the most recent kernel() call


def _token_tiles(K):
    return [(t0, min(TT, K - t0)) for t0 in range(0, K, TT)]


def _build_graph(K):
    """Build the SPMD Bass graph for capacity-K expert MLP on one core."""
    import concourse.bass as bass  # noqa: F401
    import concourse.tile as tile
    from concourse import bacc, mybir

    bf = mybir.dt.bfloat16
    f32 = mybir.dt.float32
    AF = mybir.ActivationFunctionType

    nc = bacc.Bacc("TRN2", target_bir_lowering=False, debug=False,
                   num_devices=NCORES)

    xT_d = nc.declare_dram_parameter("xT", [P, 8, K], bf, False)
    w0_d = nc.declare_dram_parameter("w0", [P, 8, 2048], bf, False)
    w1_d = nc.declare_dram_parameter("w1", [P, 16, 1024], bf, False)
    w2_d = nc.declare_dram_parameter("w2", [P, 8, 512], bf, False)
    w3_d = nc.declare_dram_parameter("w3", [P, 4, 1], bf, False)
    out_d = nc.declare_dram_parameter("out", [1, K], f32, True)

    tiles = _token_tiles(K)

    with tile.TileContext(nc) as tc:
        with (
            tc.tile_pool(name="wpool", bufs=1) as wpool,
            tc.tile_pool(name="xpool", bufs=2) as xpool,
            tc.tile_pool(name="hpool", bufs=2) as hpool,
            tc.tile_pool(name="opool", bufs=1) as opool,
            tc.tile_pool(name="psum", bufs=6, space="PSUM") as psum,
            tc.tile_pool(name="psum1", bufs=2, space="PSUM") as psum1,
        ):
            # Weights resident in SBUF for the whole kernel. Chunked DMAs so
            # the first matmuls can start before everything has landed.
            w0t = []
            for oc in range(4):  # each chunk covers out cols [oc*512, +512)
                t = wpool.tile([P, 8, 512], bf, tag=f"w0_{oc}", name=f"w0_{oc}")
                nc.sync.dma_start(t[:], w0_d[:, :, oc * 512:(oc + 1) * 512])
                w0t.append(t)
            xt = []
            for ti, (t0, tsz) in enumerate(tiles):
                t = xpool.tile([P, 8, TT], bf, tag="xt", name=f"x_{ti}")[:, :, :tsz]
                nc.sync.dma_start(t[:], xT_d[:, :, t0:t0 + tsz])
                xt.append(t)
            w1t = []
            for oc in range(2):
                t = wpool.tile([P, 16, 512], bf, tag=f"w1_{oc}", name=f"w1_{oc}")
                nc.sync.dma_start(t[:], w1_d[:, :, oc * 512:(oc + 1) * 512])
                w1t.append(t)
            w2t = wpool.tile([P, 8, 512], bf, tag="w2", name="w2")
            nc.sync.dma_start(w2t[:], w2_d[:])
            w3t = wpool.tile([P, 4, 1], bf, tag="w3", name="w3")
            nc.sync.dma_start(w3t[:], w3_d[:])

            outs = opool.tile([1, K], f32, tag="outs", name="outs")

            for ti, (t0, tsz) in enumerate(tiles):
                # L0: 1024 -> 2048, relu
                h1 = hpool.tile([P, 16, TT], bf, tag="h1", name=f"h1_{ti}")
                for o in range(16):
                    ps = psum.tile([P, TT], f32, tag="ps", name=f"ps0_{ti}_{o}")[:, :tsz]
                    w = w0t[o // 4]
                    for i in range(8):
                        nc.tensor.matmul(
                            ps, w[:, i, (o % 4) * P:(o % 4 + 1) * P],
                            xt[ti][:, i, :],
                            start=(i == 0), stop=(i == 7))
                    nc.scalar.activation(h1[:, o, :tsz], ps, AF.Relu)
                # L1: 2048 -> 1024, relu
                h2 = hpool.tile([P, 8, TT], bf, tag="h2", name=f"h2_{ti}")
                for o in range(8):
                    ps = psum.tile([P, TT], f32, tag="ps", name=f"ps1_{ti}_{o}")[:, :tsz]
                    w = w1t[o // 4]
                    for i in range(16):
                        nc.tensor.matmul(
                            ps, w[:, i, (o % 4) * P:(o % 4 + 1) * P],
                            h1[:, i, :tsz],
                            start=(i == 0), stop=(i == 15))
                    nc.scalar.activation(h2[:, o, :tsz], ps, AF.Relu)
                # L2: 1024 -> 512, relu
                h3 = hpool.tile([P, 4, TT], bf, tag="h3", name=f"h3_{ti}")
                for o in range(4):
                    ps = psum.tile([P, TT], f32, tag="ps", name=f"ps2_{ti}_{o}")[:, :tsz]
                    for i in range(8):
                        nc.tensor.matmul(
                            ps, w2t[:, i, o * P:(o + 1) * P],
                            h2[:, i, :tsz],
                            start=(i == 0), stop=(i == 7))
                    nc.scalar.activation(h3[:, o, :tsz], ps, AF.Relu)
                # L3: 512 -> 1, sigmoid
                ps = psum1.tile([1, TT], f32, tag="ps3", name=f"ps3_{ti}")[:, :tsz]
                for i in range(4):
                    nc.tensor.matmul(
                        ps, w3t[:, i, :], h3[:, i, :tsz],
                        start=(i == 0), stop=(i == 3))
                nc.scalar.activation(outs[:, t0:t0 + tsz], ps, AF.Sigmoid)

            nc.sync.dma_start(out_d[:], outs[:])

    nc.finalize()
    return nc


def _feature_major(a2d):
    """[T, F] f32/bf16 -> SBUF layout [128, F//128, T] bf16 (contiguous)."""
    T, F = a2d.shape
    a = np.ascontiguousarray(a2d.T.reshape(F // P, P, T).transpose(1, 0, 2))
    return a.astype(_BF16) if a.dtype != _BF16 else a


def kernel(x, center, w0_0, w0_1, w0_2, w0_3, wc_0, wc_1, wc_2, wc_3):
    from concourse.bass_utils import run_bass_kernel_spmd

    x = np.asarray(x, dtype=np.float32)
    center = np.asarray(center, dtype=np.float32)
    w0s = [np.asarray(w, dtype=np.float32) for w in (w0_0, w0_1, w0_2, w0_3)]
    wcs = [np.asarray(w, dtype=np.float32) for w in (wc_0, wc_1, wc_2, wc_3)]

    # --- host-side router + dispatch ---
    router = np.argmax(x @ center.T, axis=1)
    idxs = [np.where(router == c)[0] for c in range(C)]
    max_cnt = max(len(ix) for ix in idxs)
    K = max(P, int(math.ceil(max_cnt / P)) * P)

    if K not in _graph_cache:
        _graph_cache[K] = _build_graph(K)
    nc = _graph_cache[K]

    in_maps = []
    for c in range(C):
        ix = idxs[c]
        xg = np.zeros((K, DIMS), np.float32)
        xg[:len(ix)] = x[ix]
        m = {"xT": _feature_major(xg)}
        for li in range(4):
            wg = w0s[li] * wcs[li][c]
            m[f"w{li}"] = _feature_major(wg.T)  # [in,out]->[out,in]->fm [128,in/128,out]
        in_maps.append(m)

    try:
        res = run_bass_kernel_spmd(nc, in_maps, core_ids=list(range(NCORES)))
    except ModuleNotFoundError:
        # Axon stub without the NTFF profile hook: retry without tracing.
        os.environ["BASS_NEVER_TRACE"] = "1"
        res = run_bass_kernel_spmd(nc, in_maps, core_ids=list(range(NCORES)))
    global last_run
    last_run = res

    out = np.zeros(B, np.float32)
    for c in range(C):
        ix = idxs[c]
        out[ix] = res.results[c]["out"][0, :len(ix)]
    return out


# revision 7
# speedup vs baseline: 1.2069x; 1.2069x over previous
"""AdaptDHM MoE-routing kernel for one TRN2 chip (8 NeuronCores).

Strategy (expert-parallel dispatch, done host-side):
  - router = argmax(x @ center.T) picks one of C=8 clusters per token.
  - The reference computes ALL 8 cluster towers for every token and then
    gathers the selected one; only 1/8 of that work is observable. We
    dispatch each token to the core owning its cluster and run the
    4-layer MLP (1024->2048->1024->512->1, relu/sigmoid) once per token.
  - Core d receives the tokens routed to cluster d, padded to a common
    capacity K (SPMD: all cores run the same NEFF), plus the gated
    weights w0_l * wc_l[d] pre-cast to bf16 in a DMA-friendly blocked
    layout.
  - On-device: feature-major layout ([feature, token]) throughout, weights
    are the stationary matmul operand, activations the moving operand,
    fp32 PSUM accumulation, relu/sigmoid fused on the Scalar engine.
  - Host scatters per-core results back to the [B] output.
"""

import math
import os

import ml_dtypes
import numpy as np

B, DIMS = 8192, 1024
FCN = [DIMS, 2048, 1024, 512, 1]
C = 8
NCORES = 8
P = 128
TT = 512  # max token tile (matmul moving free dim / PSUM bank)

_BF16 = ml_dtypes.bfloat16

_graph_cache = {}
last_run = None  # BassKernelResults of the most recent kernel() call


def _token_tiles(K):
    """Split K into near-equal tiles of size <= TT (multiples of 8)."""
    nt = max(1, math.ceil(K / TT))
    base = K // nt
    tiles = []
    t0 = 0
    for i in range(nt):
        tsz = base + (1 if i < K - base * nt else 0)
        tiles.append((t0, tsz))
        t0 += tsz
    assert t0 == K
    return tiles


# per-layer (in_blocks, out_blocks)
_LAYER_BLOCKS = [(8, 16), (16, 8), (8, 4), (4, 1)]


def _build_graph(K):
    """Build the SPMD Bass graph for capacity-K expert MLP on one core."""
    import concourse.bass as bass  # noqa: F401
    import concourse.tile as tile
    from concourse import bacc, mybir

    bf = mybir.dt.bfloat16
    f32 = mybir.dt.float32
    AF = mybir.ActivationFunctionType

    nc = bacc.Bacc("TRN2", target_bir_lowering=False, debug=False,
                   num_devices=NCORES)

    xT_d = nc.declare_dram_parameter("xT", [P, 8, K], bf, False)
    # weights in o-block-major layout: [out_blocks, 128, in_blocks, 128]
    w_d = []
    for li, (ib, ob) in enumerate(_LAYER_BLOCKS):
        ocols = P if li < 3 else 1
        w_d.append(nc.declare_dram_parameter(
            f"w{li}", [ob, P, ib, ocols], bf, False))
    out_d = nc.declare_dram_parameter("out", [1, K], f32, True)

    tiles = _token_tiles(K)
    nt = len(tiles)

    with tile.TileContext(nc) as tc:
        with (
            tc.tile_pool(name="wpool", bufs=1) as wpool,
            tc.tile_pool(name="xpool", bufs=nt) as xpool,
            tc.tile_pool(name="hpool", bufs=2) as hpool,
            tc.tile_pool(name="opool", bufs=1) as opool,
            tc.tile_pool(name="psum", bufs=6, space="PSUM") as psum,
            tc.tile_pool(name="psum1", bufs=2, space="PSUM") as psum1,
        ):
            # --- DMAs, emitted in first-need order ---
            wt = [[None] * ob for (ib, ob) in _LAYER_BLOCKS]

            def load_wblock(li, o):
                ib, ob = _LAYER_BLOCKS[li]
                ocols = P if li < 3 else 1
                t = wpool.tile([P, ib, ocols], bf, tag=f"w{li}_{o}",
                               name=f"w{li}_{o}")
                nc.sync.dma_start(t[:], w_d[li][o])
                wt[li][o] = t

            load_wblock(0, 0)
            xt = []
            for ti, (t0, tsz) in enumerate(tiles):
                t = xpool.tile([P, 8, tsz], bf, tag=f"xt_{ti}", name=f"x_{ti}")
                nc.sync.dma_start(t[:], xT_d[:, :, t0:t0 + tsz])
                xt.append(t)
            for li in range(4):
                ib, ob = _LAYER_BLOCKS[li]
                for o in range(ob):
                    if wt[li][o] is None:
                        load_wblock(li, o)

            outs = opool.tile([1, K], f32, tag="outs", name="outs")

            for ti, (t0, tsz) in enumerate(tiles):
                # L0: 1024 -> 2048, relu
                h1 = hpool.tile([P, 16, TT], bf, tag="h1", name=f"h1_{ti}")
                for o in range(16):
                    ps = psum.tile([P, TT], f32, tag="ps", name=f"ps0_{ti}_{o}")[:, :tsz]
                    for i in range(8):
                        nc.tensor.matmul(ps, wt[0][o][:, i, :], xt[ti][:, i, :],
                                         start=(i == 0), stop=(i == 7))
                    nc.scalar.activation(h1[:, o, :tsz], ps, AF.Relu)
                # L1: 2048 -> 1024, relu
                h2 = hpool.tile([P, 8, TT], bf, tag="h2", name=f"h2_{ti}")
                for o in range(8):
                    ps = psum.tile([P, TT], f32, tag="ps", name=f"ps1_{ti}_{o}")[:, :tsz]
                    for i in range(16):
                        nc.tensor.matmul(ps, wt[1][o][:, i, :], h1[:, i, :tsz],
                                         start=(i == 0), stop=(i == 15))
                    nc.scalar.activation(h2[:, o, :tsz], ps, AF.Relu)
                # L2: 1024 -> 512, relu
                h3 = hpool.tile([P, 4, TT], bf, tag="h3", name=f"h3_{ti}")
                for o in range(4):
                    ps = psum.tile([P, TT], f32, tag="ps", name=f"ps2_{ti}_{o}")[:, :tsz]
                    for i in range(8):
                        nc.tensor.matmul(ps, wt[2][o][:, i, :], h2[:, i, :tsz],
                                         start=(i == 0), stop=(i == 7))
                    nc.scalar.activation(h3[:, o, :tsz], ps, AF.Relu)
                # L3: 512 -> 1, sigmoid
                ps = psum1.tile([1, TT], f32, tag="ps3", name=f"ps3_{ti}")[:, :tsz]
                for i in range(4):
                    nc.tensor.matmul(ps, wt[3][0][:, i, :], h3[:, i, :tsz],
                                     start=(i == 0), stop=(i == 3))
                nc.scalar.activation(outs[:, t0:t0 + tsz], ps, AF.Sigmoid)

            nc.sync.dma_start(out_d[:], outs[:])

    nc.finalize()
    return nc


def _feature_major(a2d):
    """[T, F] -> SBUF layout [128, F//128, T] bf16 (contiguous)."""
    T, F = a2d.shape
    a = np.ascontiguousarray(a2d.T.reshape(F // P, P, T).transpose(1, 0, 2))
    return a.astype(_BF16) if a.dtype != _BF16 else a


def _weight_blocked(wg):
    """[in, out] -> [out_blocks, 128, in_blocks, out_cols] bf16 contiguous,
    where out_cols = 128 (or `out` if out < 128)."""
    fin, fout = wg.shape
    ocols = min(P, fout)
    fm = _feature_major(wg.T)  # [128, in_blocks, out]
    ib = fin // P
    a = fm.reshape(P, ib, fout // ocols, ocols).transpose(2, 0, 1, 3)
    return np.ascontiguousarray(a)


def kernel(x, center, w0_0, w0_1, w0_2, w0_3, wc_0, wc_1, wc_2, wc_3):
    from concourse.bass_utils import run_bass_kernel_spmd

    x = np.asarray(x, dtype=np.float32)
    center = np.asarray(center, dtype=np.float32)
    w0s = [np.asarray(w, dtype=np.float32) for w in (w0_0, w0_1, w0_2, w0_3)]
    wcs = [np.asarray(w, dtype=np.float32) for w in (wc_0, wc_1, wc_2, wc_3)]

    # --- host-side router + dispatch ---
    router = np.argmax(x @ center.T, axis=1)
    idxs = [np.where(router == c)[0] for c in range(C)]
    max_cnt = max(len(ix) for ix in idxs)
    K = max(P, int(math.ceil(max_cnt / 8)) * 8)

    if K not in _graph_cache:
        _graph_cache[K] = _build_graph(K)
    nc = _graph_cache[K]

    in_maps = []
    for c in range(C):
        ix = idxs[c]
        xg = np.zeros((K, DIMS), np.float32)
        xg[:len(ix)] = x[ix]
        m = {"xT": _feature_major(xg)}
        for li in range(4):
            m[f"w{li}"] = _weight_blocked(w0s[li] * wcs[li][c])
        in_maps.append(m)

    try:
        res = run_bass_kernel_spmd(nc, in_maps, core_ids=list(range(NCORES)))
    except ModuleNotFoundError:
        # Axon stub without the NTFF profile hook: retry without tracing.
        os.environ["BASS_NEVER_TRACE"] = "1"
        res = run_bass_kernel_spmd(nc, in_maps, core_ids=list(range(NCORES)))
    global last_run
    last_run = res

    out = np.zeros(B, np.float32)
    for c in range(C):
        ix = idxs[c]
        out[ix] = res.results[c]["out"][0, :len(ix)]
    return out


# revision 10
# speedup vs baseline: 1.2190x; 1.0100x over previous
"""AdaptDHM MoE-routing kernel for one TRN2 chip (8 NeuronCores).

Strategy (expert-parallel dispatch, done host-side):
  - router = argmax(x @ center.T) picks one of C=8 clusters per token.
  - The reference computes ALL 8 cluster towers for every token and then
    gathers the selected one; only 1/8 of that work is observable. We
    dispatch each token to the core owning its cluster and run the
    4-layer MLP (1024->2048->1024->512->1, relu/sigmoid) once per token.
  - Core d receives the tokens routed to cluster d, padded to a common
    capacity K (SPMD: all cores run the same NEFF), plus the gated
    weights w0_l * wc_l[d] pre-cast to bf16 in a DMA-friendly blocked
    layout.
  - On-device: feature-major layout ([feature, token]) throughout, weights
    are the stationary matmul operand, activations the moving operand,
    fp32 PSUM accumulation, relu/sigmoid fused on the Scalar engine.
  - Host scatters per-core results back to the [B] output.
"""

import math
import os

import ml_dtypes
import numpy as np

B, DIMS = 8192, 1024
FCN = [DIMS, 2048, 1024, 512, 1]
C = 8
NCORES = 8
P = 128
TT = 512  # max token tile (matmul moving free dim / PSUM bank)

_BF16 = ml_dtypes.bfloat16

_graph_cache = {}
last_run = None  # BassKernelResults of the most recent kernel() call


def _token_tiles(K):
    """Split K into near-equal tiles of size <= TT (multiples of 8)."""
    nt = max(1, math.ceil(K / TT))
    base = K // nt
    tiles = []
    t0 = 0
    for i in range(nt):
        tsz = base + (1 if i < K - base * nt else 0)
        tiles.append((t0, tsz))
        t0 += tsz
    assert t0 == K
    return tiles


# per-layer (in_blocks, out_blocks)
_LAYER_BLOCKS = [(8, 16), (16, 8), (8, 4), (4, 1)]


def _build_graph(K):
    """Build the SPMD Bass graph for capacity-K expert MLP on one core."""
    import concourse.bass as bass  # noqa: F401
    import concourse.tile as tile
    from concourse import bacc, mybir

    bf = mybir.dt.bfloat16
    f32 = mybir.dt.float32
    AF = mybir.ActivationFunctionType

    nc = bacc.Bacc("TRN2", target_bir_lowering=False, debug=False,
                   num_devices=NCORES)

    xT_d = nc.declare_dram_parameter("xT", [P, 8, K], bf, False)
    # weights in o-block-major layout: [out_blocks, 128, in_blocks, 128]
    w_d = []
    for li, (ib, ob) in enumerate(_LAYER_BLOCKS):
        ocols = P if li < 3 else 1
        w_d.append(nc.declare_dram_parameter(
            f"w{li}", [ob, P, ib, ocols], bf, False))
    out_d = nc.declare_dram_parameter("out", [1, K], f32, True)

    tiles = _token_tiles(K)
    nt = len(tiles)

    with tile.TileContext(nc) as tc:
        with (
            tc.tile_pool(name="wpool", bufs=1) as wpool,
            tc.tile_pool(name="xpool", bufs=nt) as xpool,
            tc.tile_pool(name="hpool", bufs=2) as hpool,
            tc.tile_pool(name="opool", bufs=1) as opool,
            tc.tile_pool(name="psum", bufs=6, space="PSUM") as psum,
            tc.tile_pool(name="psum1", bufs=2, space="PSUM") as psum1,
        ):
            # --- DMAs, emitted in first-need order ---
            wt = [[None] * ob for (ib, ob) in _LAYER_BLOCKS]

            def load_wblock(li, o):
                ib, ob = _LAYER_BLOCKS[li]
                ocols = P if li < 3 else 1
                t = wpool.tile([P, ib, ocols], bf, tag=f"w{li}_{o}",
                               name=f"w{li}_{o}")
                nc.sync.dma_start(t[:], w_d[li][o])
                wt[li][o] = t

            def load_xtile(ti, per_i=False):
                t0, tsz = tiles[ti]
                if per_i:
                    # separate tile per input block: first matmuls can start
                    # after 1/8 of the tile's DMA
                    sub = []
                    for i in range(8):
                        t = xpool.tile([P, 1, tsz], bf, tag=f"xt_{ti}_{i}",
                                       name=f"x_{ti}_{i}")
                        nc.sync.dma_start(t[:], xT_d[:, i:i + 1, t0:t0 + tsz])
                        sub.append(t)
                    return sub
                t = xpool.tile([P, 8, tsz], bf, tag=f"xt_{ti}", name=f"x_{ti}")
                nc.sync.dma_start(t[:], xT_d[:, :, t0:t0 + tsz])
                return [t[:, i:i + 1, :] for i in range(8)]

            load_wblock(0, 0)
            xt = [load_xtile(0, per_i=True)]
            for li in range(4):
                ib, ob = _LAYER_BLOCKS[li]
                for o in range(ob):
                    if wt[li][o] is None:
                        load_wblock(li, o)
            for ti in range(1, nt):
                xt.append(load_xtile(ti))


            def relu(dst, src, o):
                if o % 2 == 0:
                    nc.scalar.activation(dst, src, AF.Relu)
                else:
                    nc.vector.tensor_scalar_max(dst, src, 0.0)

            outs = opool.tile([1, K], f32, tag="outs", name="outs")

            for ti, (t0, tsz) in enumerate(tiles):
                # L0: 1024 -> 2048, relu
                h1 = hpool.tile([P, 16, TT], bf, tag="h1", name=f"h1_{ti}")
                for o in range(16):
                    ps = psum.tile([P, TT], f32, tag="ps", name=f"ps0_{ti}_{o}")[:, :tsz]
                    for i in range(8):
                        nc.tensor.matmul(ps, wt[0][o][:, i, :], xt[ti][i][:, 0, :],
                                         start=(i == 0), stop=(i == 7))
                    relu(h1[:, o, :tsz], ps, o)
                # L1: 2048 -> 1024, relu
                h2 = hpool.tile([P, 8, TT], bf, tag="h2", name=f"h2_{ti}")
                for o in range(8):
                    ps = psum.tile([P, TT], f32, tag="ps", name=f"ps1_{ti}_{o}")[:, :tsz]
                    for i in range(16):
                        nc.tensor.matmul(ps, wt[1][o][:, i, :], h1[:, i, :tsz],
                                         start=(i == 0), stop=(i == 15))
                    relu(h2[:, o, :tsz], ps, o)
                # L2: 1024 -> 512, relu
                h3 = hpool.tile([P, 4, TT], bf, tag="h3", name=f"h3_{ti}")
                for o in range(4):
                    ps = psum.tile([P, TT], f32, tag="ps", name=f"ps2_{ti}_{o}")[:, :tsz]
                    for i in range(8):
                        nc.tensor.matmul(ps, wt[2][o][:, i, :], h2[:, i, :tsz],
                                         start=(i == 0), stop=(i == 7))
                    relu(h3[:, o, :tsz], ps, o)
                # L3: 512 -> 1, sigmoid
                ps = psum1.tile([1, TT], f32, tag="ps3", name=f"ps3_{ti}")[:, :tsz]
                for i in range(4):
                    nc.tensor.matmul(ps, wt[3][0][:, i, :], h3[:, i, :tsz],
                                     start=(i == 0), stop=(i == 3))
                nc.scalar.activation(outs[:, t0:t0 + tsz], ps, AF.Sigmoid)

            nc.sync.dma_start(out_d[:], outs[:])

    nc.finalize()
    return nc


def _feature_major(a2d):
    """[T, F] -> SBUF layout [128, F//128, T] bf16 (contiguous)."""
    T, F = a2d.shape
    a = np.ascontiguousarray(a2d.T.reshape(F // P, P, T).transpose(1, 0, 2))
    return a.astype(_BF16) if a.dtype != _BF16 else a


def _weight_blocked(wg):
    """[in, out] -> [out_blocks, 128, in_blocks, out_cols] bf16 contiguous,
    where out_cols = 128 (or `out` if out < 128)."""
    fin, fout = wg.shape
    ocols = min(P, fout)
    fm = _feature_major(wg.T)  # [128, in_blocks, out]
    ib = fin // P
    a = fm.reshape(P, ib, fout // ocols, ocols).transpose(2, 0, 1, 3)
    return np.ascontiguousarray(a)


def kernel(x, center, w0_0, w0_1, w0_2, w0_3, wc_0, wc_1, wc_2, wc_3):
    from concourse.bass_utils import run_bass_kernel_spmd

    x = np.asarray(x, dtype=np.float32)
    center = np.asarray(center, dtype=np.float32)
    w0s = [np.asarray(w, dtype=np.float32) for w in (w0_0, w0_1, w0_2, w0_3)]
    wcs = [np.asarray(w, dtype=np.float32) for w in (wc_0, wc_1, wc_2, wc_3)]

    # --- host-side router + dispatch ---
    router = np.argmax(x @ center.T, axis=1)
    idxs = [np.where(router == c)[0] for c in range(C)]
    max_cnt = max(len(ix) for ix in idxs)
    K = max(P, int(math.ceil(max_cnt / 8)) * 8)

    if K not in _graph_cache:
        _graph_cache[K] = _build_graph(K)
    nc = _graph_cache[K]

    in_maps = []
    for c in range(C):
        ix = idxs[c]
        xg = np.zeros((K, DIMS), np.float32)
        xg[:len(ix)] = x[ix]
        m = {"xT": _feature_major(xg)}
        for li in range(4):
            m[f"w{li}"] = _weight_blocked(w0s[li] * wcs[li][c])
        in_maps.append(m)

    try:
        res = run_bass_kernel_spmd(nc, in_maps, core_ids=list(range(NCORES)))
    except ModuleNotFoundError:
        # Axon stub without the NTFF profile hook: retry without tracing.
        os.environ["BASS_NEVER_TRACE"] = "1"
        res = run_bass_kernel_spmd(nc, in_maps, core_ids=list(range(NCORES)))
    global last_run
    last_run = res

    out = np.zeros(B, np.float32)
    for c in range(C):
        ix = idxs[c]
        out[ix] = res.results[c]["out"][0, :len(ix)]
    return out


# revision 12
# speedup vs baseline: 3.2298x; 2.6496x over previous
"""AdaptDHM MoE-routing kernel for one TRN2 chip (8 NeuronCores).

Strategy (expert-parallel dispatch, done host-side):
  - router = argmax(x @ center.T) picks one of C=8 clusters per token.
  - The reference computes ALL 8 cluster towers for every token and then
    gathers the selected one; only 1/8 of that work is observable. We
    dispatch each token to the core owning its cluster and run the
    4-layer MLP (1024->2048->1024->512->1, relu/sigmoid) once per token.
  - Core d receives the tokens routed to cluster d, padded to a common
    capacity K (SPMD: all cores run the same NEFF), plus the gated
    weights w0_l * wc_l[d] in a DMA-friendly blocked layout.
  - Compute: layers 0-2 in fp8-e4m3 with DoubleRow matmuls (2x TensorE
    rate), layer 3 in bf16; fp32 PSUM accumulation throughout. Inputs and
    weights are pre-scaled into fp8's normal range; the inverse scales are
    folded into the relu/copy that writes each layer's activations.
  - On-device: feature-major layout ([feature, token]), weights stationary,
    activations moving, relu split across Scalar+Vector engines.
  - Host scatters per-core results back to the [B] output.
"""

import math
import os

import ml_dtypes
import numpy as np

B, DIMS = 8192, 1024
FCN = [DIMS, 2048, 1024, 512, 1]
C = 8
NCORES = 8
P = 128
TT = 512  # max token tile (matmul moving free dim / PSUM bank)

_BF16 = ml_dtypes.bfloat16

_graph_cache = {}
last_run = None  # BassKernelResults of the most recent kernel() call

# per-layer (in_blocks, out_blocks)
_LAYER_BLOCKS = [(8, 16), (16, 8), (8, 4), (4, 1)]


def _token_tiles(K):
    """Split K into near-equal tiles of size <= TT (multiples of 16)."""
    assert K % 16 == 0
    nt = max(1, math.ceil(K / TT))
    units = K // 16
    base = units // nt
    tiles = []
    t0 = 0
    for i in range(nt):
        u = base + (1 if i < units - base * nt else 0)
        tiles.append((t0, u * 16))
        t0 += u * 16
    assert t0 == K
    return tiles


def _build_graph(K, c0, c1, c2):
    """Build the SPMD Bass graph for capacity-K expert MLP on one core.

    c0..c2 are the descale factors folded into each layer's activation
    write (product of the input/weight pre-scales for that layer).
    """
    import concourse.bass as bass  # noqa: F401
    import concourse.tile as tile
    from concourse import bacc, mybir

    f8 = mybir.dt.float8e4
    bf = mybir.dt.bfloat16
    f32 = mybir.dt.float32
    AF = mybir.ActivationFunctionType
    DR = mybir.MatmulPerfMode.DoubleRow
    wdt = [f8, f8, f8, bf]

    nc = bacc.Bacc("TRN2", target_bir_lowering=False, debug=False,
                   num_devices=NCORES)

    xT_d = nc.declare_dram_parameter("xT", [P, 8, K], f8, False)
    # weights in o-block-major layout: [out_blocks, 128, in_blocks, out_cols]
    w_d = []
    for li, (ib, ob) in enumerate(_LAYER_BLOCKS):
        ocols = P if li < 3 else 1
        w_d.append(nc.declare_dram_parameter(
            f"w{li}", [ob, P, ib, ocols], wdt[li], False))
    out_d = nc.declare_dram_parameter("out", [1, K], f32, True)

    tiles = _token_tiles(K)
    nt = len(tiles)

    with tile.TileContext(nc) as tc:
        with (
            tc.tile_pool(name="wpool", bufs=1) as wpool,
            tc.tile_pool(name="xpool", bufs=nt) as xpool,
            tc.tile_pool(name="hpool", bufs=2) as hpool,
            tc.tile_pool(name="opool", bufs=1) as opool,
            tc.tile_pool(name="psum", bufs=6, space="PSUM") as psum,
            tc.tile_pool(name="psum1", bufs=2, space="PSUM") as psum1,
        ):
            # --- DMAs, emitted in first-need order ---
            wt = [[None] * ob for (ib, ob) in _LAYER_BLOCKS]

            def load_wblock(li, o):
                ib, ob = _LAYER_BLOCKS[li]
                ocols = P if li < 3 else 1
                t = wpool.tile([P, ib, ocols], wdt[li], tag=f"w{li}_{o}",
                               name=f"w{li}_{o}")
                nc.sync.dma_start(t[:], w_d[li][o])
                wt[li][o] = t

            def load_xtile(ti, per_pair=False):
                t0, tsz = tiles[ti]
                if per_pair:
                    # separate tile per DoubleRow input pair: first matmuls
                    # can start after 1/4 of the tile's DMA
                    sub = []
                    for k in range(4):
                        t = xpool.tile([P, 2, tsz], f8, tag=f"xt_{ti}_{k}",
                                       name=f"x_{ti}_{k}")
                        nc.sync.dma_start(t[:], xT_d[:, 2 * k:2 * k + 2,
                                                     t0:t0 + tsz])
                        sub.append(t[:])
                    return sub
                t = xpool.tile([P, 8, tsz], f8, tag=f"xt_{ti}", name=f"x_{ti}")
                nc.sync.dma_start(t[:], xT_d[:, :, t0:t0 + tsz])
                return [t[:, 2 * k:2 * k + 2, :] for k in range(4)]

            load_wblock(0, 0)
            xt = [load_xtile(0, per_pair=True)]
            for li in range(4):
                ib, ob = _LAYER_BLOCKS[li]
                for o in range(ob):
                    if wt[li][o] is None:
                        load_wblock(li, o)
            for ti in range(1, nt):
                xt.append(load_xtile(ti))

            def relu(dst, src, o, scale):
                # alternate engines; both apply the descale then clamp at 0
                if o % 2 == 0:
                    nc.scalar.activation(dst, src, AF.Relu, scale=scale)
                else:
                    nc.vector.tensor_scalar(dst, src, scale, 0.0,
                                            mybir.AluOpType.mult,
                                            mybir.AluOpType.max)

            outs = opool.tile([1, K], f32, tag="outs", name="outs")

            for ti, (t0, tsz) in enumerate(tiles):
                # L0: 1024 -> 2048 fp8 DoubleRow, relu
                h1 = hpool.tile([P, 16, TT], f8, tag="h1", name=f"h1_{ti}")
                for o in range(16):
                    ps = psum.tile([P, TT], f32, tag="ps", name=f"ps0_{ti}_{o}")[:, :tsz]
                    for k in range(4):
                        nc.tensor.matmul(ps, wt[0][o][:, 2 * k:2 * k + 2, :],
                                         xt[ti][k], start=(k == 0),
                                         stop=(k == 3), perf_mode=DR)
                    relu(h1[:, o, :tsz], ps, o, c0)
                # L1: 2048 -> 1024 fp8 DoubleRow, relu
                h2 = hpool.tile([P, 8, TT], f8, tag="h2", name=f"h2_{ti}")
                for o in range(8):
                    ps = psum.tile([P, TT], f32, tag="ps", name=f"ps1_{ti}_{o}")[:, :tsz]
                    for k in range(8):
                        nc.tensor.matmul(ps, wt[1][o][:, 2 * k:2 * k + 2, :],
                                         h1[:, 2 * k:2 * k + 2, :tsz],
                                         start=(k == 0), stop=(k == 7),
                                         perf_mode=DR)
                    relu(h2[:, o, :tsz], ps, o, c1)
                # L2: 1024 -> 512 fp8 DoubleRow, relu -> bf16
                h3 = hpool.tile([P, 4, TT], bf, tag="h3", name=f"h3_{ti}")
                for o in range(4):
                    ps = psum.tile([P, TT], f32, tag="ps", name=f"ps2_{ti}_{o}")[:, :tsz]
                    for k in range(4):
                        nc.tensor.matmul(ps, wt[2][o][:, 2 * k:2 * k + 2, :],
                                         h2[:, 2 * k:2 * k + 2, :tsz],
                                         start=(k == 0), stop=(k == 3),
                                         perf_mode=DR)
                    relu(h3[:, o, :tsz], ps, o, c2)
                # L3: 512 -> 1 bf16, sigmoid
                ps = psum1.tile([1, TT], f32, tag="ps3", name=f"ps3_{ti}")[:, :tsz]
                for i in range(4):
                    nc.tensor.matmul(ps, wt[3][0][:, i, :], h3[:, i, :tsz],
                                     start=(i == 0), stop=(i == 3))
                nc.scalar.activation(outs[:, t0:t0 + tsz], ps, AF.Sigmoid)

            nc.sync.dma_start(out_d[:], outs[:])

    nc.finalize()
    return nc


def _np_dt(mdt_name):
    from concourse import mybir
    return mybir.dt.np(getattr(mybir.dt, mdt_name))


def _feature_major(a2d, npdt):
    """[T, F] -> SBUF layout [128, F//128, T] (contiguous)."""
    T, F = a2d.shape
    a = np.ascontiguousarray(a2d.T.reshape(F // P, P, T).transpose(1, 0, 2))
    return a.astype(npdt)


def _weight_blocked(wg, npdt):
    """[in, out] -> [out_blocks, 128, in_blocks, out_cols] contiguous."""
    fin, fout = wg.shape
    ocols = min(P, fout)
    # blk[ob, p, i, oc] = wg[i*128+p, ob*ocols+oc]
    a = wg.reshape(fin // P, P, fout // ocols, ocols).transpose(2, 1, 0, 3)
    return np.ascontiguousarray(a).astype(npdt)


def kernel(x, center, w0_0, w0_1, w0_2, w0_3, wc_0, wc_1, wc_2, wc_3):
    from concourse.bass_utils import run_bass_kernel_spmd

    x = np.asarray(x, dtype=np.float32)
    center = np.asarray(center, dtype=np.float32)
    w0s = [np.asarray(w, dtype=np.float32) for w in (w0_0, w0_1, w0_2, w0_3)]
    wcs = [np.asarray(w, dtype=np.float32) for w in (wc_0, wc_1, wc_2, wc_3)]

    # --- host-side router + dispatch ---
    router = np.argmax(x @ center.T, axis=1)
    idxs = [np.where(router == c)[0] for c in range(C)]
    max_cnt = max(len(ix) for ix in idxs)
    K = max(P, int(math.ceil(max_cnt / 16)) * 16)

    # gated weights per cluster, and global per-layer fp8 pre-scales
    wg = [[w0s[li] * wcs[li][c] for c in range(C)] for li in range(4)]
    FP8_MAX = 240.0
    ws = [max(np.abs(wg[li][c]).max() for c in range(C)) / FP8_MAX
          for li in range(3)]
    hs0 = np.abs(x).max() / FP8_MAX

    # estimate activation ranges on a sample to pick gains G1, G2 that keep
    # stored fp8 activations well inside the normal range
    smp = x[:: max(1, B // 512)]
    m1 = m2 = 1e-9
    for c in range(C):
        a1 = np.maximum(smp @ wg[0][c], 0)
        m1 = max(m1, a1.max())
        a2 = np.maximum(a1 @ wg[1][c], 0)
        m2 = max(m2, a2.max())
    G1 = FP8_MAX / (8.0 * m1)
    G2 = FP8_MAX / (8.0 * m2)
    c0 = float(hs0 * ws[0] * G1)
    c1 = float(ws[1] * G2 / G1)
    c2 = float(ws[2] / G2)

    key = (K, round(c0, 12), round(c1, 12), round(c2, 12))
    if key not in _graph_cache:
        _graph_cache[key] = _build_graph(K, c0, c1, c2)
    nc = _graph_cache[key]

    f8np = _np_dt("float8e4")
    bfnp = _np_dt("bfloat16")
    in_maps = []
    for c in range(C):
        ix = idxs[c]
        xg = np.zeros((K, DIMS), np.float32)
        xg[:len(ix)] = x[ix] / hs0
        m = {"xT": _feature_major(xg, f8np)}
        for li in range(3):
            m[f"w{li}"] = _weight_blocked(wg[li][c] / ws[li], f8np)
        m["w3"] = _weight_blocked(wg[3][c], bfnp)
        in_maps.append(m)

    try:
        res = run_bass_kernel_spmd(nc, in_maps, core_ids=list(range(NCORES)))
    except ModuleNotFoundError:
        # Axon stub without the NTFF profile hook: retry without tracing.
        os.environ["BASS_NEVER_TRACE"] = "1"
        res = run_bass_kernel_spmd(nc, in_maps, core_ids=list(range(NCORES)))
    global last_run
    last_run = res

    out = np.zeros(B, np.float32)
    for c in range(C):
        ix = idxs[c]
        out[ix] = res.results[c]["out"][0, :len(ix)]
    return out


# revision 18
# speedup vs baseline: 3.6529x; 1.1310x over previous
"""AdaptDHM MoE-routing kernel for one TRN2 chip (8 NeuronCores).

Strategy (expert-parallel dispatch, done host-side):
  - router = argmax(x @ center.T) picks one of C=8 clusters per token.
  - The reference computes ALL 8 cluster towers for every token and then
    gathers the selected one; only 1/8 of that work is observable. We
    dispatch each token to the core owning its cluster and run the
    4-layer MLP (1024->2048->1024->512->1, relu/sigmoid) once per token.
  - Core d receives the tokens routed to cluster d, padded to a common
    capacity K (SPMD: all cores run the same NEFF), plus the gated
    weights w0_l * wc_l[d] in a DMA-friendly blocked layout.
  - Compute: layers 0-2 in fp8-e4m3 with DoubleRow matmuls (2x TensorE
    rate), layer 3 in bf16; fp32 PSUM accumulation throughout. Inputs and
    weights are pre-scaled into fp8's normal range; the inverse scales are
    folded into the relu/copy that writes each layer's activations.
  - On-device: feature-major layout ([feature, token]), weights stationary,
    activations moving, relu split across Scalar+Vector engines.
  - Host scatters per-core results back to the [B] output.
"""

import math
import os

import ml_dtypes
import numpy as np

B, DIMS = 8192, 1024
FCN = [DIMS, 2048, 1024, 512, 1]
C = 8
NCORES = 8
P = 128
TT = 512  # max token tile (matmul moving free dim / PSUM bank)

_BF16 = ml_dtypes.bfloat16

_graph_cache = {}
last_run = None  # BassKernelResults of the most recent kernel() call

# per-layer (in_blocks, out_blocks)
_LAYER_BLOCKS = [(8, 16), (16, 8), (8, 4), (4, 1)]
# out columns per DMA-able weight block (~256KB fp8 each)
_WBLK_OCOLS = [256, 128, 512, 1]


def _token_tiles(K):
    """Split K into near-equal tiles of size <= TT (multiples of 16)."""
    assert K % 16 == 0
    nt = max(1, math.ceil(K / TT))
    units = K // 16
    base = units // nt
    tiles = []
    t0 = 0
    for i in range(nt):
        u = base + (1 if i < units - base * nt else 0)
        tiles.append((t0, u * 16))
        t0 += u * 16
    assert t0 == K
    return tiles


def _build_graph(K, c0, c1, c2):
    """Build the SPMD Bass graph for capacity-K expert MLP on one core.

    c0..c2 are the descale factors folded into each layer's activation
    write (product of the input/weight pre-scales for that layer).
    """
    import concourse.bass as bass  # noqa: F401
    import concourse.tile as tile
    from concourse import bacc, mybir

    f8 = mybir.dt.float8e4
    bf = mybir.dt.bfloat16
    f32 = mybir.dt.float32
    AF = mybir.ActivationFunctionType
    DR = mybir.MatmulPerfMode.DoubleRow
    wdt = [f8, f8, f8, bf]

    nc = bacc.Bacc("TRN2", target_bir_lowering=False, debug=False,
                   num_devices=NCORES)

    xT_d = nc.declare_dram_parameter("xT", [P, 8, K], f8, False)
    # weights in o-block-major layout: [n_blocks, 128, in_blocks, blk_ocols]
    w_d = []
    for li, (ib, ob) in enumerate(_LAYER_BLOCKS):
        ocols = _WBLK_OCOLS[li]
        nblk = (ob * P) // ocols if li < 3 else 1
        w_d.append(nc.declare_dram_parameter(
            f"w{li}", [nblk, P, ib, ocols], wdt[li], False))
    out_d = nc.declare_dram_parameter("out", [1, K], f32, True)

    tiles = _token_tiles(K)
    nt = len(tiles)

    with tile.TileContext(nc) as tc:
        with (
            tc.tile_pool(name="wpool", bufs=1) as wpool,
            tc.tile_pool(name="xpool", bufs=nt) as xpool,
            tc.tile_pool(name="hpool", bufs=2) as hpool,
            tc.tile_pool(name="opool", bufs=1) as opool,
            tc.tile_pool(name="psum", bufs=7, space="PSUM") as psum,
            tc.tile_pool(name="psum1", bufs=1, space="PSUM") as psum1,
        ):
            # --- DMAs, emitted in first-need order ---
            wblk = [[None] * ((ob * P) // _WBLK_OCOLS[li] if li < 3 else 1)
                    for li, (ib, ob) in enumerate(_LAYER_BLOCKS)]

            def load_wblock(li, blk):
                ib, ob = _LAYER_BLOCKS[li]
                ocols = _WBLK_OCOLS[li]
                t = wpool.tile([P, ib, ocols], wdt[li], tag=f"w{li}_{blk}",
                               name=f"w{li}_{blk}")
                nc.sync.dma_start(t[:], w_d[li][blk])
                wblk[li][blk] = t

            def wslice(li, o, k2):
                """lhsT AP for out 128-block o, DoubleRow pair k2."""
                opb = _WBLK_OCOLS[li] // P  # 128-out-blocks per dma block
                t = wblk[li][o // opb]
                off = (o % opb) * P
                return t[:, 2 * k2:2 * k2 + 2, off:off + P]

            def load_xtile(ti, split=False):
                t0, tsz = tiles[ti]
                if split:
                    # two half tiles: first matmuls start after half the DMA
                    sub = []
                    for hf in range(2):
                        t = xpool.tile([P, 4, tsz], f8, tag=f"xt_{ti}_{hf}",
                                       name=f"x_{ti}_{hf}")
                        nc.sync.dma_start(t[:], xT_d[:, 4 * hf:4 * hf + 4,
                                                     t0:t0 + tsz])
                        sub += [t[:, 0:2, :], t[:, 2:4, :]]
                    return sub
                t = xpool.tile([P, 8, tsz], f8, tag=f"xt_{ti}", name=f"x_{ti}")
                nc.sync.dma_start(t[:], xT_d[:, :, t0:t0 + tsz])
                return [t[:, 2 * k:2 * k + 2, :] for k in range(4)]

            load_wblock(0, 0)
            xt = [load_xtile(0, split=True)]
            for li in range(4):
                for blk in range(len(wblk[li])):
                    if wblk[li][blk] is None:
                        load_wblock(li, blk)
            for ti in range(1, nt):
                xt.append(load_xtile(ti))

            def relu(dst, src, o, scale):
                # alternate engines; both apply the descale then clamp at 0
                if o % 2 == 0:
                    nc.scalar.activation(dst, src, AF.Relu, scale=scale)
                else:
                    nc.vector.tensor_scalar(dst, src, scale, 0.0,
                                            mybir.AluOpType.mult,
                                            mybir.AluOpType.max)

            outs = opool.tile([1, K], f32, tag="outs", name="outs")

            for ti, (t0, tsz) in enumerate(tiles):
                # L0: 1024 -> 2048 fp8 DoubleRow, relu
                h1 = hpool.tile([P, 16, TT], f8, tag="h1", name=f"h1_{ti}")
                for o in range(16):
                    ps = psum.tile([P, TT], f32, tag="ps", name=f"ps0_{ti}_{o}")[:, :tsz]
                    for k in range(4):
                        nc.tensor.matmul(ps, wslice(0, o, k),
                                         xt[ti][k], start=(k == 0),
                                         stop=(k == 3), perf_mode=DR)
                    relu(h1[:, o, :tsz], ps, o, c0)
                # L1: 2048 -> 1024 fp8 DoubleRow, relu
                h2 = hpool.tile([P, 8, TT], f8, tag="h2", name=f"h2_{ti}")
                for o in range(8):
                    ps = psum.tile([P, TT], f32, tag="ps", name=f"ps1_{ti}_{o}")[:, :tsz]
                    for k in range(8):
                        nc.tensor.matmul(ps, wslice(1, o, k),
                                         h1[:, 2 * k:2 * k + 2, :tsz],
                                         start=(k == 0), stop=(k == 7),
                                         perf_mode=DR)
                    relu(h2[:, o, :tsz], ps, o, c1)
                # L2: 1024 -> 512 fp8 DoubleRow, relu -> bf16
                h3 = hpool.tile([P, 4, TT], bf, tag="h3", name=f"h3_{ti}")
                for o in range(4):
                    ps = psum.tile([P, TT], f32, tag="ps", name=f"ps2_{ti}_{o}")[:, :tsz]
                    for k in range(4):
                        nc.tensor.matmul(ps, wslice(2, o, k),
                                         h2[:, 2 * k:2 * k + 2, :tsz],
                                         start=(k == 0), stop=(k == 3),
                                         perf_mode=DR)
                    relu(h3[:, o, :tsz], ps, o, c2)
                # L3: 512 -> 1 bf16, sigmoid
                ps = psum1.tile([1, TT], f32, tag="ps3", name=f"ps3_{ti}")[:, :tsz]
                for i in range(4):
                    nc.tensor.matmul(ps, wblk[3][0][:, i, :], h3[:, i, :tsz],
                                     start=(i == 0), stop=(i == 3))
                nc.scalar.activation(outs[:, t0:t0 + tsz], ps, AF.Sigmoid)

            nc.sync.dma_start(out_d[:], outs[:])

    nc.finalize()
    return nc


def _np_dt(mdt_name):
    from concourse import mybir
    return mybir.dt.np(getattr(mybir.dt, mdt_name))


def _feature_major(a2d, npdt):
    """[T, F] -> SBUF layout [128, F//128, T] (contiguous)."""
    T, F = a2d.shape
    a = np.ascontiguousarray(a2d.T.reshape(F // P, P, T).transpose(1, 0, 2))
    return a.astype(npdt)


def _weight_blocked(wg, npdt, ocols):
    """[in, out] -> [n_blocks, 128, in_blocks, ocols] contiguous."""
    fin, fout = wg.shape
    ocols = min(ocols, fout)
    # blk[ob, p, i, oc] = wg[i*128+p, ob*ocols+oc]
    a = wg.reshape(fin // P, P, fout // ocols, ocols).transpose(2, 1, 0, 3)
    return np.ascontiguousarray(a).astype(npdt)


def kernel(x, center, w0_0, w0_1, w0_2, w0_3, wc_0, wc_1, wc_2, wc_3):
    from concourse.bass_utils import run_bass_kernel_spmd

    x = np.asarray(x, dtype=np.float32)
    center = np.asarray(center, dtype=np.float32)
    w0s = [np.asarray(w, dtype=np.float32) for w in (w0_0, w0_1, w0_2, w0_3)]
    wcs = [np.asarray(w, dtype=np.float32) for w in (wc_0, wc_1, wc_2, wc_3)]

    # --- host-side router + dispatch ---
    router = np.argmax(x @ center.T, axis=1)
    idxs = [np.where(router == c)[0] for c in range(C)]
    max_cnt = max(len(ix) for ix in idxs)
    K = max(P, int(math.ceil(max_cnt / 16)) * 16)

    # gated weights per cluster, and global per-layer fp8 pre-scales
    wg = [[w0s[li] * wcs[li][c] for c in range(C)] for li in range(4)]
    FP8_MAX = 240.0
    ws = [max(np.abs(wg[li][c]).max() for c in range(C)) / FP8_MAX
          for li in range(3)]
    hs0 = np.abs(x).max() / FP8_MAX

    # estimate activation ranges on a sample to pick gains G1, G2 that keep
    # stored fp8 activations well inside the normal range
    smp = x[:: max(1, B // 512)]
    m1 = m2 = 1e-9
    for c in range(C):
        a1 = np.maximum(smp @ wg[0][c], 0)
        m1 = max(m1, a1.max())
        a2 = np.maximum(a1 @ wg[1][c], 0)
        m2 = max(m2, a2.max())
    G1 = FP8_MAX / (8.0 * m1)
    G2 = FP8_MAX / (8.0 * m2)
    c0 = float(hs0 * ws[0] * G1)
    c1 = float(ws[1] * G2 / G1)
    c2 = float(ws[2] / G2)

    key = (K, round(c0, 12), round(c1, 12), round(c2, 12))
    if key not in _graph_cache:
        _graph_cache[key] = _build_graph(K, c0, c1, c2)
    nc = _graph_cache[key]

    f8np = _np_dt("float8e4")
    bfnp = _np_dt("bfloat16")
    in_maps = []
    for c in range(C):
        ix = idxs[c]
        xg = np.zeros((K, DIMS), np.float32)
        xg[:len(ix)] = x[ix] / hs0
        m = {"xT": _feature_major(xg, f8np)}
        for li in range(3):
            m[f"w{li}"] = _weight_blocked(wg[li][c] / ws[li], f8np, _WBLK_OCOLS[li])
        m["w3"] = _weight_blocked(wg[3][c], bfnp, _WBLK_OCOLS[3])
        in_maps.append(m)

    try:
        res = run_bass_kernel_spmd(nc, in_maps, core_ids=list(range(NCORES)))
    except ModuleNotFoundError:
        # Axon stub without the NTFF profile hook: retry without tracing.
        os.environ["BASS_NEVER_TRACE"] = "1"
        res = run_bass_kernel_spmd(nc, in_maps, core_ids=list(range(NCORES)))
    global last_run
    last_run = res

    out = np.zeros(B, np.float32)
    for c in range(C):
        ix = idxs[c]
        out[ix] = res.results[c]["out"][0, :len(ix)]
    return out


# revision 24
# speedup vs baseline: 3.6552x; 1.0006x over previous
"""AdaptDHM MoE-routing kernel for one TRN2 chip (8 NeuronCores).

Strategy (expert-parallel dispatch, done host-side):
  - router = argmax(x @ center.T) picks one of C=8 clusters per token.
  - The reference computes ALL 8 cluster towers for every token and then
    gathers the selected one; only 1/8 of that work is observable. We
    dispatch each token to the core owning its cluster and run the
    4-layer MLP (1024->2048->1024->512->1, relu/sigmoid) once per token.
  - Core d receives the tokens routed to cluster d, padded to a common
    capacity K (SPMD: all cores run the same NEFF), plus the gated
    weights w0_l * wc_l[d] in a DMA-friendly blocked layout.
  - Compute: layers 0-2 in fp8-e4m3 with DoubleRow matmuls (2x TensorE
    rate), layer 3 in bf16; fp32 PSUM accumulation throughout. Inputs and
    weights are pre-scaled into fp8's normal range; the inverse scales are
    folded into the relu/copy that writes each layer's activations.
  - On-device: feature-major layout ([feature, token]), weights stationary,
    activations moving, relu split across Scalar+Vector engines.
  - Host scatters per-core results back to the [B] output.
"""

import math
import os

import ml_dtypes
import numpy as np

B, DIMS = 8192, 1024
FCN = [DIMS, 2048, 1024, 512, 1]
C = 8
NCORES = 8
P = 128
TT = 512  # max token tile (matmul moving free dim / PSUM bank)

_BF16 = ml_dtypes.bfloat16

_graph_cache = {}
last_run = None  # BassKernelResults of the most recent kernel() call

# per-layer (in_blocks, out_blocks)
_LAYER_BLOCKS = [(8, 16), (16, 8), (8, 4), (4, 1)]
# out columns per DMA-able weight block (~256KB fp8 each)
_WBLK_OCOLS = [256, 128, 512, 1]


def _token_tiles(K):
    """Split K into near-equal tiles of size <= TT (multiples of 16)."""
    assert K % 16 == 0
    nt = max(1, math.ceil(K / TT))
    units = K // 16
    base = units // nt
    tiles = []
    t0 = 0
    for i in range(nt):
        u = base + (1 if i < units - base * nt else 0)
        tiles.append((t0, u * 16))
        t0 += u * 16
    assert t0 == K
    return tiles


def _build_graph(K, c0, c1, c2):
    """Build the SPMD Bass graph for capacity-K expert MLP on one core.

    c0..c2 are the descale factors folded into each layer's activation
    write (product of the input/weight pre-scales for that layer).
    """
    import concourse.bass as bass  # noqa: F401
    import concourse.tile as tile
    from concourse import bacc, mybir

    f8 = mybir.dt.float8e4
    bf = mybir.dt.bfloat16
    f32 = mybir.dt.float32
    AF = mybir.ActivationFunctionType
    DR = mybir.MatmulPerfMode.DoubleRow
    wdt = [f8, f8, f8, bf]

    nc = bacc.Bacc("TRN2", target_bir_lowering=False, debug=False,
                   num_devices=NCORES)

    xT_d = nc.declare_dram_parameter("xT", [P, 8, K], f8, False)
    # weights in o-block-major layout: [n_blocks, 128, in_blocks, blk_ocols]
    w_d = []
    for li, (ib, ob) in enumerate(_LAYER_BLOCKS):
        ocols = _WBLK_OCOLS[li]
        nblk = (ob * P) // ocols if li < 3 else 1
        w_d.append(nc.declare_dram_parameter(
            f"w{li}", [nblk, P, ib, ocols], wdt[li], False))
    out_d = nc.declare_dram_parameter("out", [1, K], f32, True)

    tiles = _token_tiles(K)
    nt = len(tiles)

    with tile.TileContext(nc) as tc:
        with (
            tc.tile_pool(name="wpool", bufs=1) as wpool,
            tc.tile_pool(name="xpool", bufs=1) as xpool,
            tc.tile_pool(name="hpool", bufs=3) as hpool,
            tc.tile_pool(name="opool", bufs=1) as opool,
            tc.tile_pool(name="psum", bufs=7, space="PSUM") as psum,
            tc.tile_pool(name="psum1", bufs=1, space="PSUM") as psum1,
        ):
            # --- DMAs, emitted in first-need order ---
            wblk = [[None] * ((ob * P) // _WBLK_OCOLS[li] if li < 3 else 1)
                    for li, (ib, ob) in enumerate(_LAYER_BLOCKS)]

            def load_wblock(li, blk):
                ib, ob = _LAYER_BLOCKS[li]
                ocols = _WBLK_OCOLS[li]
                t = wpool.tile([P, ib, ocols], wdt[li], tag=f"w{li}_{blk}",
                               name=f"w{li}_{blk}")
                nc.sync.dma_start(t[:], w_d[li][blk])
                wblk[li][blk] = t

            def wslice(li, o, k2):
                """lhsT AP for out 128-block o, DoubleRow pair k2."""
                opb = _WBLK_OCOLS[li] // P  # 128-out-blocks per dma block
                t = wblk[li][o // opb]
                off = (o % opb) * P
                return t[:, 2 * k2:2 * k2 + 2, off:off + P]

            def load_xtile(ti, split=False):
                t0, tsz = tiles[ti]
                if split:
                    # pair 0 in its own small DMA so the first matmul can
                    # start as early as possible; pairs 1-3 in one DMA
                    ta = xpool.tile([P, 2, tsz], f8, tag=f"xt_{ti}_a",
                                    name=f"x_{ti}_a")
                    nc.sync.dma_start(ta[:], xT_d[:, 0:2, t0:t0 + tsz])
                    tb = xpool.tile([P, 6, tsz], f8, tag=f"xt_{ti}_b",
                                    name=f"x_{ti}_b")
                    nc.sync.dma_start(tb[:], xT_d[:, 2:8, t0:t0 + tsz])
                    return [ta[:], tb[:, 0:2, :], tb[:, 2:4, :], tb[:, 4:6, :]]
                t = xpool.tile([P, 8, tsz], f8, tag=f"xt_{ti}", name=f"x_{ti}")
                nc.sync.dma_start(t[:], xT_d[:, :, t0:t0 + tsz])
                return [t[:, 2 * k:2 * k + 2, :] for k in range(4)]

            load_wblock(0, 0)
            xt = [load_xtile(0, split=True)]
            for li in range(4):
                for blk in range(len(wblk[li])):
                    if wblk[li][blk] is None:
                        load_wblock(li, blk)
            for ti in range(1, nt):
                xt.append(load_xtile(ti))

            def relu(dst, src, o, scale):
                # alternate engines; both apply the descale then clamp at 0
                if o % 2 == 0:
                    nc.scalar.activation(dst, src, AF.Relu, scale=scale)
                else:
                    nc.vector.tensor_scalar(dst, src, scale, 0.0,
                                            mybir.AluOpType.mult,
                                            mybir.AluOpType.max)

            outs = opool.tile([1, K], f32, tag="outs", name="outs")

            for ti, (t0, tsz) in enumerate(tiles):
                # L0: 1024 -> 2048 fp8 DoubleRow, relu
                h1 = hpool.tile([P, 16, TT], f8, tag="h1", name=f"h1_{ti}")
                for o in range(16):
                    ps = psum.tile([P, TT], f32, tag="ps", name=f"ps0_{ti}_{o}")[:, :tsz]
                    for k in range(4):
                        nc.tensor.matmul(ps, wslice(0, o, k),
                                         xt[ti][k], start=(k == 0),
                                         stop=(k == 3), perf_mode=DR)
                    relu(h1[:, o, :tsz], ps, o, c0)
                # L1: 2048 -> 1024 fp8 DoubleRow, relu
                h2 = hpool.tile([P, 8, TT], f8, tag="h2", name=f"h2_{ti}")
                for o in range(8):
                    ps = psum.tile([P, TT], f32, tag="ps", name=f"ps1_{ti}_{o}")[:, :tsz]
                    for k in range(8):
                        nc.tensor.matmul(ps, wslice(1, o, k),
                                         h1[:, 2 * k:2 * k + 2, :tsz],
                                         start=(k == 0), stop=(k == 7),
                                         perf_mode=DR)
                    relu(h2[:, o, :tsz], ps, o, c1)
                # L2: 1024 -> 512 fp8 DoubleRow, relu -> bf16
                h3 = hpool.tile([P, 4, TT], bf, tag="h3", name=f"h3_{ti}")
                for o in range(4):
                    ps = psum.tile([P, TT], f32, tag="ps", name=f"ps2_{ti}_{o}")[:, :tsz]
                    for k in range(4):
                        nc.tensor.matmul(ps, wslice(2, o, k),
                                         h2[:, 2 * k:2 * k + 2, :tsz],
                                         start=(k == 0), stop=(k == 3),
                                         perf_mode=DR)
                    relu(h3[:, o, :tsz], ps, o, c2)
                # L3: 512 -> 1 bf16, sigmoid
                ps = psum1.tile([1, TT], f32, tag="ps3", name=f"ps3_{ti}")[:, :tsz]
                for i in range(4):
                    nc.tensor.matmul(ps, wblk[3][0][:, i, :], h3[:, i, :tsz],
                                     start=(i == 0), stop=(i == 3))
                nc.scalar.activation(outs[:, t0:t0 + tsz], ps, AF.Sigmoid)

            nc.sync.dma_start(out_d[:], outs[:])

    nc.finalize()
    return nc


def _np_dt(mdt_name):
    from concourse import mybir
    return mybir.dt.np(getattr(mybir.dt, mdt_name))


def _feature_major(a2d, npdt):
    """[T, F] -> SBUF layout [128, F//128, T] (contiguous)."""
    T, F = a2d.shape
    a = np.ascontiguousarray(a2d.T.reshape(F // P, P, T).transpose(1, 0, 2))
    return a.astype(npdt)


def _weight_blocked(wg, npdt, ocols):
    """[in, out] -> [n_blocks, 128, in_blocks, ocols] contiguous."""
    fin, fout = wg.shape
    ocols = min(ocols, fout)
    # blk[ob, p, i, oc] = wg[i*128+p, ob*ocols+oc]
    a = wg.reshape(fin // P, P, fout // ocols, ocols).transpose(2, 1, 0, 3)
    return np.ascontiguousarray(a).astype(npdt)


def kernel(x, center, w0_0, w0_1, w0_2, w0_3, wc_0, wc_1, wc_2, wc_3):
    from concourse.bass_utils import run_bass_kernel_spmd

    x = np.asarray(x, dtype=np.float32)
    center = np.asarray(center, dtype=np.float32)
    w0s = [np.asarray(w, dtype=np.float32) for w in (w0_0, w0_1, w0_2, w0_3)]
    wcs = [np.asarray(w, dtype=np.float32) for w in (wc_0, wc_1, wc_2, wc_3)]

    # --- host-side router + dispatch ---
    router = np.argmax(x @ center.T, axis=1)
    idxs = [np.where(router == c)[0] for c in range(C)]
    max_cnt = max(len(ix) for ix in idxs)
    K = max(P, int(math.ceil(max_cnt / 16)) * 16)

    # gated weights per cluster, and global per-layer fp8 pre-scales
    wg = [[w0s[li] * wcs[li][c] for c in range(C)] for li in range(4)]
    FP8_MAX = 240.0
    TINY = 1e-30
    ws = [max(TINY, max(np.abs(wg[li][c]).max() for c in range(C))) / FP8_MAX
          for li in range(3)]
    hs0 = max(TINY, np.abs(x).max()) / FP8_MAX

    # estimate activation ranges on a sample to pick gains G1, G2 that keep
    # stored fp8 activations well inside the normal range
    smp = x[:: max(1, B // 512)]
    m1 = m2 = 1e-9
    for c in range(C):
        a1 = np.maximum(smp @ wg[0][c], 0)
        m1 = max(m1, a1.max())
        a2 = np.maximum(a1 @ wg[1][c], 0)
        m2 = max(m2, a2.max())
    G1 = FP8_MAX / (8.0 * m1)
    G2 = FP8_MAX / (8.0 * m2)
    c0 = float(hs0 * ws[0] * G1)
    c1 = float(ws[1] * G2 / G1)
    c2 = float(ws[2] / G2)

    key = (K, round(c0, 12), round(c1, 12), round(c2, 12))
    if key not in _graph_cache:
        _graph_cache[key] = _build_graph(K, c0, c1, c2)
    nc = _graph_cache[key]

    f8np = _np_dt("float8e4")
    bfnp = _np_dt("bfloat16")
    in_maps = []
    for c in range(C):
        ix = idxs[c]
        xg = np.zeros((K, DIMS), np.float32)
        xg[:len(ix)] = x[ix] / hs0
        m = {"xT": _feature_major(xg, f8np)}
        for li in range(3):
            m[f"w{li}"] = _weight_blocked(wg[li][c] / ws[li], f8np, _WBLK_OCOLS[li])
        m["w3"] = _weight_blocked(wg[3][c], bfnp, _WBLK_OCOLS[3])
        in_maps.append(m)

    try:
        res = run_bass_kernel_spmd(nc, in_maps, core_ids=list(range(NCORES)))
    except ModuleNotFoundError:
        # Axon stub without the NTFF profile hook: retry without tracing.
        os.environ["BASS_NEVER_TRACE"] = "1"
        res = run_bass_kernel_spmd(nc, in_maps, core_ids=list(range(NCORES)))
    global last_run
    last_run = res

    out = np.zeros(B, np.float32)
    for c in range(C):
        ix = idxs[c]
        out[ix] = res.results[c]["out"][0, :len(ix)]
    return out

